# revision 5
# baseline (speedup 1.0000x reference)
# Trainium2 Bass kernel for ComputePartialCharges (segment_reduce).
#
# Math (per molecule m over its atoms i, segment_ids sorted):
#   inv_h = 1/h ;  lam_m = (sum(inv_h*e) + sum(fc)) / sum(inv_h)
#   q_i = (lam_m - e_i) * inv_h_i
#
# Strategy: data-parallel over 8 NeuronCores. The atom stream is cut at
# molecule boundaries into SLOTS of up to F atoms (8 cores x NT tiles x 128
# partitions slots, right-padded), so every molecule lives entirely inside one
# (core, tile, partition) slot. On device, per-molecule sums become SEGMENTED
# SCANS along the free dimension (tensor_tensor_scan with the run-boundary
# mask as the carry gate) — no gathers/scatters, no cross-core communication:
#   d0[t]   = (seg[t] == seg[t-1])            boundary mask
#   S       = seg-scan(d0, inv_h*e + fc)      run-prefix numerator
#   B       = seg-scan(d0, inv_h)             run-prefix denominator
#   Bm      = d0shift*BIG + B                 ~inf except at run ends
#   lam_m   = S * (1/Bm)                      lam at run ends, ~0 elsewhere
#   lam     = reversed seg-scan(d0shift, lam_m)   propagate lam to whole run
#   q       = (lam - e) * inv_h
import os
import sys

import numpy as np

if "JAX_PLATFORMS" not in os.environ:
    # bass2jax under axon needs the axon jax platform; leave default alone.
    pass

for _p in ("/opt/trn_rl_repo", "/root/.axon_site/_ro/trn_rl_repo"):
    if _p not in sys.path and os.path.isdir(_p):
        sys.path.append(_p)

import concourse.bacc as bacc
import concourse.bass as bass
import concourse.mybir as mybir
import concourse.tile as tile
from concourse.bass_utils import run_bass_kernel_spmd

N_CORES = 8
P = 128          # SBUF partitions
F = 2048         # atoms per slot (free dim)
BIG = 1.0e30

# Filled by kernel() on each call; test harness reads exec_time_ns from here.
_last_results = None


def _build_program(n_tiles: int, f: int, k_loop: int = 1) -> bass.Bass:
    """One NeuronCore's program; identical on all cores (SPMD).

    k_loop > 1 repeats the whole pass (same data) — used only by the timing
    harness to amortize host-side dispatch overhead out of measurements.
    """
    nc = bacc.Bacc("TRN2", target_bir_lowering=False, debug=False)
    AL = mybir.AluOpType
    x = nc.dram_tensor("x", [n_tiles, P, 2 * f], mybir.dt.float32,
                       kind="ExternalInput")
    seg = nc.dram_tensor("seg", [n_tiles, P, f], mybir.dt.int16,
                         kind="ExternalInput")
    fc = nc.dram_tensor("fc", [n_tiles, P, f], mybir.dt.int8,
                        kind="ExternalInput")
    q = nc.dram_tensor("q", [n_tiles, P, f], mybir.dt.float32,
                       kind="ExternalOutput")

    with tile.TileContext(nc) as tc:
        with tc.tile_pool(name="p", bufs=2) as pool:
            for t in [ti for _ in range(k_loop) for ti in range(n_tiles)]:
                x_t = pool.tile([P, 2 * f], mybir.dt.float32, tag="x")
                seg_t = pool.tile([P, f], mybir.dt.int16, tag="seg")
                fc_t = pool.tile([P, f], mybir.dt.int8, tag="fc")
                nc.sync.dma_start(x_t[:], x.ap()[t])
                nc.sync.dma_start(seg_t[:], seg.ap()[t])
                nc.sync.dma_start(fc_t[:], fc.ap()[t])
                e = x_t[:, 0:2 * f:2]
                h = x_t[:, 1:2 * f:2]

                inv_h = pool.tile([P, f], mybir.dt.float32, tag="inv_h")
                nc.vector.reciprocal(inv_h[:], h)

                d0 = pool.tile([P, f + 1], mybir.dt.bfloat16, tag="d0")
                nc.vector.tensor_tensor(out=d0[:, 1:f], in0=seg_t[:, 1:f],
                                        in1=seg_t[:, 0:f - 1], op=AL.is_equal)
                nc.vector.memset(d0[:, 0:1], 0.0)
                nc.vector.memset(d0[:, f:f + 1], 0.0)

                v1 = pool.tile([P, f], mybir.dt.float32, tag="AB")
                nc.vector.tensor_tensor(out=v1[:], in0=e, in1=inv_h[:],
                                        op=AL.mult)
                w = pool.tile([P, f], mybir.dt.float32, tag="w")
                nc.vector.tensor_tensor(out=w[:], in0=v1[:], in1=fc_t[:],
                                        op=AL.add)
                S = pool.tile([P, f], mybir.dt.float32, tag="SL")
                nc.vector.tensor_tensor_scan(out=S[:], data0=d0[:, 0:f],
                                             data1=w[:], initial=0.0,
                                             op0=AL.mult, op1=AL.add)
                B = pool.tile([P, f], mybir.dt.float32, tag="BT")
                nc.vector.tensor_tensor_scan(out=B[:], data0=d0[:, 0:f],
                                             data1=inv_h[:], initial=0.0,
                                             op0=AL.mult, op1=AL.add)
                Bm = pool.tile([P, f], mybir.dt.float32, tag="AB")
                nc.vector.scalar_tensor_tensor(out=Bm[:], in0=d0[:, 1:f + 1],
                                               scalar=BIG, in1=B[:],
                                               op0=AL.mult, op1=AL.add)
                Rm = pool.tile([P, f], mybir.dt.float32, tag="Rm")
                nc.vector.reciprocal(Rm[:], Bm[:])
                lam_m = pool.tile([P, f], mybir.dt.float32, tag="LQ")
                nc.vector.tensor_tensor(out=lam_m[:], in0=S[:], in1=Rm[:],
                                        op=AL.mult)
                lam = pool.tile([P, f], mybir.dt.float32, tag="SL")
                rev = lambda ap: ap[:, ::-1]
                nc.vector.tensor_tensor_scan(out=rev(lam[:]),
                                             data0=rev(d0[:, 1:f + 1]),
                                             data1=rev(lam_m[:]), initial=0.0,
                                             op0=AL.mult, op1=AL.add)
                t1 = pool.tile([P, f], mybir.dt.float32, tag="BT")
                nc.vector.scalar_tensor_tensor(out=t1[:], in0=e, scalar=-1.0,
                                               in1=lam[:], op0=AL.mult,
                                               op1=AL.add)
                qt = pool.tile([P, f], mybir.dt.float32, tag="LQ")
                nc.vector.tensor_tensor(out=qt[:], in0=t1[:], in1=inv_h[:],
                                        op=AL.mult)
                nc.sync.dma_start(q.ap()[t], qt[:])
    nc.compile()
    return nc


def _pack(x, segment_ids, formal_charge):
    """Cut the sorted atom stream at molecule boundaries into padded slots.

    Returns per-core input maps plus the bookkeeping needed to unpad.
    """
    n = segment_ids.shape[0]
    seg = np.ascontiguousarray(segment_ids)
    # cut points usable as slot boundaries: start of every molecule run
    bnd = np.flatnonzero(seg[1:] != seg[:-1]) + 1
    bounds = np.concatenate(([0], bnd, [n]))  # sorted cut candidates

    n_tiles = max(1, -(-n // (N_CORES * P * F)))
    while True:
        n_slots = N_CORES * n_tiles * P
        # equal-ish targets snapped DOWN to a molecule boundary
        targets = ((np.arange(1, n_slots) * n) // n_slots)
        idx = np.searchsorted(bounds, targets, side="right") - 1
        cuts = np.concatenate(([0], bounds[idx], [n]))
        cuts = np.maximum.accumulate(cuts)
        lengths = np.diff(cuts)
        if lengths.max() <= F:
            break
        n_tiles += 1  # pathological molecule/slot; retry with more capacity

    offs = cuts[:-1]
    ar = np.arange(F)
    gather = np.minimum(offs[:, None] + ar[None, :], n - 1)
    valid = ar[None, :] < lengths[:, None]

    e = x[:, 0]
    h = x[:, 1]
    seg16 = (seg.astype(np.int64) & 0xFFFF).astype(np.uint16).view(np.int16)
    # pad id differs from the slot's last real id; equal within the pad run
    last_real = np.maximum(offs + lengths - 1, offs)
    pad_fill = (((seg16[last_real].view(np.uint16).astype(np.int64) + 1)
                 & 0xFFFF).astype(np.uint16).view(np.int16))

    x_pad = np.empty((n_slots, 2 * F), np.float32)
    x_pad[:, 0::2] = np.where(valid, e[gather], np.float32(0.0))
    x_pad[:, 1::2] = np.where(valid, h[gather], np.float32(1.0))
    seg_pad = np.where(valid, seg16[gather], pad_fill[:, None])
    fc_pad = np.where(valid, formal_charge[gather], 0).astype(np.int8)

    x_pad = x_pad.reshape(N_CORES, n_tiles, P, 2 * F)
    seg_pad = seg_pad.reshape(N_CORES, n_tiles, P, F)
    fc_pad = fc_pad.reshape(N_CORES, n_tiles, P, F)

    # flat position of atom i inside the padded [n_slots*F] layout
    slot_of_atom = np.repeat(np.arange(n_slots), lengths)
    pos = slot_of_atom * F + (np.arange(n) - np.repeat(offs, lengths))
    return x_pad, seg_pad, fc_pad, n_tiles, pos


def kernel(x, segment_ids, formal_charge, num_segments):
    global _last_results
    x = np.asarray(x, dtype=np.float32)
    segment_ids = np.asarray(segment_ids, dtype=np.int32)
    formal_charge = np.asarray(formal_charge, dtype=np.int32)
    n = segment_ids.shape[0]

    x_pad, seg_pad, fc_pad, n_tiles, pos = _pack(x, segment_ids,
                                                 formal_charge)
    nc = _build_program(n_tiles, F)
    in_maps = [
        {"x": x_pad[c], "seg": seg_pad[c], "fc": fc_pad[c]}
        for c in range(N_CORES)
    ]

    if os.environ.get("CPC_SIM") == "1":  # dev-only CoreSim path
        from concourse.bass_interp import CoreSim
        results = []
        for c in range(N_CORES):
            sim = CoreSim(nc)
            for k, v in in_maps[c].items():
                sim.tensor(k)[:] = v
            sim.simulate(check_with_hw=False)
            results.append({"q": sim.tensor("q").copy()})
        _last_results = None
    else:
        res = run_bass_kernel_spmd(nc, in_maps, core_ids=list(range(N_CORES)))
        _last_results = res
        results = res.results

    q_pad = np.stack([results[c]["q"] for c in range(N_CORES)])
    q = q_pad.reshape(-1)[pos]
    return q.reshape(n, 1).astype(np.float32)


# revision 10
# speedup vs baseline: 2.0108x; 2.0108x over previous
# Trainium2 Bass kernel for ComputePartialCharges (segment_reduce).
#
# Math (per molecule m over its atoms i, segment_ids sorted):
#   inv_h = 1/h ;  lam_m = (sum(inv_h*e) + sum(fc)) / sum(inv_h)
#   q_i = (lam_m - e_i) * inv_h_i
#
# Strategy: data-parallel over 8 NeuronCores. The atom stream is cut at
# molecule boundaries into SLOTS of up to F atoms (8 cores x NT tiles x 128
# partitions slots, right-padded), so every molecule lives entirely inside one
# (core, tile, partition) slot. On device, per-molecule sums become SEGMENTED
# SCANS along the free dimension (tensor_tensor_scan with the run-boundary
# mask as the carry gate) — no gathers/scatters, no cross-core communication:
#   d0[t]   = (seg[t] == seg[t-1])            boundary mask
#   S       = seg-scan(d0, inv_h*e + fc)      run-prefix numerator
#   B       = seg-scan(d0, inv_h)             run-prefix denominator
#   Bm      = d0shift*BIG + B                 ~inf except at run ends
#   lam_m   = S * (1/Bm)                      lam at run ends, ~0 elsewhere
#   lam     = reversed seg-scan(d0shift, lam_m)   propagate lam to whole run
#   q       = (lam - e) * inv_h
import os
import sys

import numpy as np

if "JAX_PLATFORMS" not in os.environ:
    # bass2jax under axon needs the axon jax platform; leave default alone.
    pass

for _p in ("/opt/trn_rl_repo", "/root/.axon_site/_ro/trn_rl_repo"):
    if _p not in sys.path and os.path.isdir(_p):
        sys.path.append(_p)

import concourse.bacc as bacc
import concourse.bass as bass
import concourse.mybir as mybir
import concourse.tile as tile
from concourse.bass_utils import run_bass_kernel_spmd

N_CORES = 8
P = 128          # SBUF partitions
F = 2048         # atoms per slot (free dim)
BIG = 1.0e30

# Filled by kernel() on each call; test harness reads exec_time_ns from here.
_last_results = None


def _build_program(n_tiles: int, f: int, k_loop: int = 1) -> bass.Bass:
    """One NeuronCore's program; identical on all cores (SPMD).

    k_loop > 1 repeats the whole pass (same data) — used only by the timing
    harness to amortize host-side dispatch overhead out of measurements.
    """
    nc = bacc.Bacc("TRN2", target_bir_lowering=False, debug=False)
    AL = mybir.AluOpType
    e_d = nc.dram_tensor("e", [n_tiles, P, f], mybir.dt.float32,
                         kind="ExternalInput")
    h_d = nc.dram_tensor("h", [n_tiles, P, f], mybir.dt.float32,
                         kind="ExternalInput")
    seg = nc.dram_tensor("seg", [n_tiles, P, f], mybir.dt.int16,
                         kind="ExternalInput")
    fc = nc.dram_tensor("fc", [n_tiles, P, f], mybir.dt.int8,
                        kind="ExternalInput")
    q = nc.dram_tensor("q", [n_tiles, P, f], mybir.dt.float32,
                       kind="ExternalOutput")

    with tile.TileContext(nc) as tc:
        with (tc.tile_pool(name="ld3", bufs=3) as ld3,
              tc.tile_pool(name="p2", bufs=2) as p2):
            for t in [ti for _ in range(k_loop) for ti in range(n_tiles)]:
                e_t = ld3.tile([P, f], mybir.dt.float32, tag="e")
                h_t = p2.tile([P, f], mybir.dt.float32, tag="h")
                seg_t = ld3.tile([P, f], mybir.dt.int16, tag="seg")
                fc_t = ld3.tile([P, f], mybir.dt.int8, tag="fc")
                nc.sync.dma_start(e_t[:], e_d.ap()[t])
                nc.sync.dma_start(h_t[:], h_d.ap()[t])
                nc.sync.dma_start(seg_t[:], seg.ap()[t])
                nc.sync.dma_start(fc_t[:], fc.ap()[t])

                inv_h = p2.tile([P, f], mybir.dt.float32, tag="inv_h")
                scr = p2.tile([P, f], mybir.dt.float32, tag="scr")
                nc.vector.reciprocal_approx_accurate(inv_h[:], h_t[:],
                                                     scratch=scr[:])

                d0 = p2.tile([P, f + 1], mybir.dt.bfloat16, tag="d0")
                nc.vector.tensor_tensor(out=d0[:, 1:f], in0=seg_t[:, 1:f],
                                        in1=seg_t[:, 0:f - 1], op=AL.is_equal)
                nc.vector.memset(d0[:, 0:1], 0.0)
                nc.vector.memset(d0[:, f:f + 1], 0.0)

                # v1 = e*inv_h, then in-place v1 += fc
                v1 = p2.tile([P, f], mybir.dt.float32, tag="v1")
                nc.vector.tensor_tensor(out=v1[:], in0=e_t[:], in1=inv_h[:],
                                        op=AL.mult)
                nc.vector.tensor_tensor(out=v1[:], in0=v1[:], in1=fc_t[:],
                                        op=AL.add)
                S = p2.tile([P, f], mybir.dt.float32, tag="S")
                nc.vector.tensor_tensor_scan(out=S[:], data0=d0[:, 0:f],
                                             data1=v1[:], initial=0.0,
                                             op0=AL.mult, op1=AL.add)
                B = p2.tile([P, f], mybir.dt.float32, tag="B")
                nc.vector.tensor_tensor_scan(out=B[:], data0=d0[:, 0:f],
                                             data1=inv_h[:], initial=0.0,
                                             op0=AL.mult, op1=AL.add)
                # in-place: B := d0shift*BIG + B  (~inf except at run ends)
                nc.vector.scalar_tensor_tensor(out=B[:], in0=d0[:, 1:f + 1],
                                               scalar=BIG, in1=B[:],
                                               op0=AL.mult, op1=AL.add)
                Rm = p2.tile([P, f], mybir.dt.float32, tag="Rm")
                nc.vector.reciprocal_approx_fast(Rm[:], B[:])
                # in-place: S := S*Rm  (lam at run ends, ~0 elsewhere)
                nc.vector.tensor_tensor(out=S[:], in0=S[:], in1=Rm[:],
                                        op=AL.mult)
                lam = p2.tile([P, f], mybir.dt.float32, tag="lam")
                rev = lambda ap: ap[:, ::-1]
                nc.vector.tensor_tensor_scan(out=rev(lam[:]),
                                             data0=rev(d0[:, 1:f + 1]),
                                             data1=rev(S[:]), initial=0.0,
                                             op0=AL.mult, op1=AL.add)
                # in-place: lam := -e + lam ; lam := lam*inv_h
                nc.vector.scalar_tensor_tensor(out=lam[:], in0=e_t[:],
                                               scalar=-1.0, in1=lam[:],
                                               op0=AL.mult, op1=AL.add)
                nc.vector.tensor_tensor(out=lam[:], in0=lam[:], in1=inv_h[:],
                                        op=AL.mult)
                nc.sync.dma_start(q.ap()[t], lam[:])
    nc.compile()
    return nc


def _pack(x, segment_ids, formal_charge):
    """Cut the sorted atom stream at molecule boundaries into padded slots.

    Returns per-core input maps plus the bookkeeping needed to unpad.
    """
    n = segment_ids.shape[0]
    seg = np.ascontiguousarray(segment_ids)
    # cut points usable as slot boundaries: start of every molecule run
    bnd = np.flatnonzero(seg[1:] != seg[:-1]) + 1
    bounds = np.concatenate(([0], bnd, [n]))  # sorted cut candidates

    n_tiles = max(1, -(-n // (N_CORES * P * F)))
    while True:
        n_slots = N_CORES * n_tiles * P
        # equal-ish targets snapped DOWN to a molecule boundary
        targets = ((np.arange(1, n_slots) * n) // n_slots)
        idx = np.searchsorted(bounds, targets, side="right") - 1
        cuts = np.concatenate(([0], bounds[idx], [n]))
        cuts = np.maximum.accumulate(cuts)
        lengths = np.diff(cuts)
        if lengths.max() <= F:
            break
        n_tiles += 1  # pathological molecule/slot; retry with more capacity

    offs = cuts[:-1]
    ar = np.arange(F)
    gather = np.minimum(offs[:, None] + ar[None, :], n - 1)
    valid = ar[None, :] < lengths[:, None]

    e = x[:, 0]
    h = x[:, 1]
    seg16 = (seg.astype(np.int64) & 0xFFFF).astype(np.uint16).view(np.int16)
    # pad id differs from the slot's last real id; equal within the pad run
    last_real = np.maximum(offs + lengths - 1, offs)
    pad_fill = (((seg16[last_real].view(np.uint16).astype(np.int64) + 1)
                 & 0xFFFF).astype(np.uint16).view(np.int16))

    e_pad = np.where(valid, e[gather], np.float32(0.0))
    h_pad = np.where(valid, h[gather], np.float32(1.0))
    seg_pad = np.where(valid, seg16[gather], pad_fill[:, None])
    fc_pad = np.where(valid, formal_charge[gather], 0).astype(np.int8)

    e_pad = e_pad.reshape(N_CORES, n_tiles, P, F)
    h_pad = h_pad.reshape(N_CORES, n_tiles, P, F)
    seg_pad = seg_pad.reshape(N_CORES, n_tiles, P, F)
    fc_pad = fc_pad.reshape(N_CORES, n_tiles, P, F)

    # flat position of atom i inside the padded [n_slots*F] layout
    slot_of_atom = np.repeat(np.arange(n_slots), lengths)
    pos = slot_of_atom * F + (np.arange(n) - np.repeat(offs, lengths))
    return e_pad, h_pad, seg_pad, fc_pad, n_tiles, pos


def kernel(x, segment_ids, formal_charge, num_segments):
    global _last_results
    x = np.asarray(x, dtype=np.float32)
    segment_ids = np.asarray(segment_ids, dtype=np.int32)
    formal_charge = np.asarray(formal_charge, dtype=np.int32)
    n = segment_ids.shape[0]

    e_pad, h_pad, seg_pad, fc_pad, n_tiles, pos = _pack(x, segment_ids,
                                                        formal_charge)
    nc = _build_program(n_tiles, F)
    in_maps = [
        {"e": e_pad[c], "h": h_pad[c], "seg": seg_pad[c], "fc": fc_pad[c]}
        for c in range(N_CORES)
    ]

    if os.environ.get("CPC_SIM") == "1":  # dev-only CoreSim path
        from concourse.bass_interp import CoreSim
        results = []
        for c in range(N_CORES):
            sim = CoreSim(nc)
            for k, v in in_maps[c].items():
                sim.tensor(k)[:] = v
            sim.simulate(check_with_hw=False)
            results.append({"q": sim.tensor("q").copy()})
        _last_results = None
    else:
        res = run_bass_kernel_spmd(nc, in_maps, core_ids=list(range(N_CORES)))
        _last_results = res
        results = res.results

    q_pad = np.stack([results[c]["q"] for c in range(N_CORES)])
    q = q_pad.reshape(-1)[pos]
    return q.reshape(n, 1).astype(np.float32)
